# revision 48
# baseline (speedup 1.0000x reference)
"""Causal self-attention Trainium2 kernel (v3, bf16).

Problem: B=8, T=2048, C=512, H=8 heads (D=64), fp32 in/out.
  q = x@Wq.T ; k = x@Wk.T ; v = x@Wv.T  (per head)
  att = softmax(mask(q k^T / sqrt(D)))  ; y = att v ; out = y@Wp.T

Sharding: data-parallel over batch B across 8 NeuronCores (one batch
element per core, weights replicated). No collectives.

Design (all matmuls bf16; PSUM f32; rel err ~4e-3):
  - Scores computed transposed: sT[kpos, q] per (head-pair, q-tile,
    k-chunk) into 2-bank PSUM tiles; ScalarE exp IS the PSUM->SBUF
    evacuation (writes bf16 P^T tiles), exact causal trim per chunk.
  - PV transposed-accumulate: out[q(128), 65] with lhsT = P^T chunk
    (stationary) and rhs = ones-augmented V chunk (65 moving cols,
    col 64 = softmax denominator) accumulated over k-chunks. Bursts
    run per (head, q-block) sequentially so each PSUM bank has at
    most ONE open accumulation group at a time (hardware constraint).
  - Denominators land per-q-partition: reciprocal + broadcast
    tensor-mult scale+evac (no DRAM round trip).
  - y transposed back for the output projection on the DMA XBAR
    (bf16 DMA transpose; PE/DVE untouched, HWDGE issue batched).
  - All projection / epilogue work is queued as (cost, deadline)
    thunks drained against each iteration's spare PE budget, with the
    final PV bursts and the scale/transpose epilogue deferred into
    the next block's exp-heavy phase, so PE and ScalarE (the two
    ~147us-busy engines) stay saturated across block boundaries.
"""

import numpy as np
import ml_dtypes

import concourse.bass as bass
import concourse.bacc as bacc
import concourse.tile as tile
from concourse import mybir
from concourse.bass_utils import run_bass_kernel_spmd

B, T, C, H = 8, 2048, 512, 8
D = C // H          # 64
NT = T // 512       # 4 q-tiles of 512
NB = T // 128       # 16 k-blocks of 128
f32 = mybir.dt.float32
bf16 = mybir.dt.bfloat16
EXP = mybir.ActivationFunctionType.Exp
N_CORES = 8
BF = ml_dtypes.bfloat16


def build_nc():
    nc = bacc.Bacc(None)
    xT = nc.dram_tensor("xT", [C, T], bf16, kind="ExternalInput")
    wq = nc.dram_tensor("wqT", [C, C], bf16, kind="ExternalInput")
    wk = nc.dram_tensor("wkT", [C, C], bf16, kind="ExternalInput")
    wv = nc.dram_tensor("wvT", [C, C], bf16, kind="ExternalInput")
    wp = nc.dram_tensor("wpT", [C, C], bf16, kind="ExternalInput")
    out = nc.dram_tensor("out", [T, C], f32, kind="ExternalOutput")

    with tile.TileContext(nc) as tc:
        with tc.tile_pool(name="const", bufs=1) as constp, \
             tc.tile_pool(name="xw", bufs=1) as xw, \
             tc.tile_pool(name="kq", bufs=1) as kqp, \
             tc.tile_pool(name="vp", bufs=1) as vpool, \
             tc.tile_pool(name="ptp", bufs=21) as ptp, \
             tc.tile_pool(name="ys", bufs=3) as ypool, \
             tc.tile_pool(name="rc", bufs=3) as rcp, \
             tc.tile_pool(name="yt", bufs=1) as ytp, \
             tc.tile_pool(name="ob", bufs=3) as otp, \
             tc.tile_pool(name="pp", bufs=2, space="PSUM") as psp, \
             tc.tile_pool(name="qk", bufs=2, space="PSUM") as qkp, \
             tc.tile_pool(name="ac", bufs=1, space="PSUM") as acp:

            # ---- constants
            tri2 = constp.tile([128, 2, 128], bf16, tag="tri", name="tri2")
            nc.gpsimd.memset(tri2[:, :, :], 1.0)
            for half in range(2):
                sl = tri2[:, half, :]
                nc.gpsimd.affine_select(
                    out=sl, in_=sl, pattern=[[1, 128]], base=0,
                    channel_multiplier=-1,
                    compare_op=mybir.AluOpType.is_ge, fill=0.0)

            # ---- loads (k/q weights + x cols 0:512 first so compute starts
            # early). One 3D-AP DMA per weight matrix / x column chunk keeps
            # the SP sequencer (565ns per DMA issue) off the critical path.
            def w_tile(name):
                t = xw.tile([128, 4, C], bf16, tag=name, name=name)
                return t, [t[:, ci, :] for ci in range(4)]

            def load_w_bulk(t, dram):
                nc.sync.dma_start(out=t[:, :, :],
                                  in_=dram[:, :].rearrange("(c p) d -> p c d", c=4))

            xtile = xw.tile([128, 4, T], bf16, tag="xT", name="xtile")
            xt = [xtile[:, ci, :] for ci in range(4)]

            def load_x_n(n):
                nc.sync.dma_start(
                    out=xtile[:, :, 512 * n:512 * (n + 1)],
                    in_=xT[:, 512 * n:512 * (n + 1)].rearrange("(c p) t -> p c t", c=4))

            wk_t, wkt = w_tile("wk")
            wq_t, wqt = w_tile("wq")
            wv_t, wvt = w_tile("wv")
            wp_t, wpt = w_tile("wp")
            # wk first, then x cols 0:512 in chunks (so the first projection
            # accumulation matmuls start per-chunk), then the rest in bulk
            # (each DMA costs ~625ns on the serialized HWDGE device).
            for ci in range(4):
                nc.sync.dma_start(out=wk_t[:, ci, :],
                                  in_=wk[128 * ci:128 * (ci + 1), :])
                nc.sync.dma_start(out=xtile[:, ci, 0:512],
                                  in_=xT[128 * ci:128 * (ci + 1), 0:512])
            load_w_bulk(wq_t, wq)
            load_w_bulk(wv_t, wv)
            load_x_n(1)
            load_x_n(2)
            load_x_n(3)
            load_w_bulk(wp_t, wp)

            kt = [kqp.tile([128, T], bf16, tag=f"kt{p}", name=f"kt{p}") for p in range(4)]
            qt = [kqp.tile([128, T], bf16, tag=f"qt{p}", name=f"qt{p}") for p in range(4)]
            ytT = [ytp.tile([128, T], bf16, tag=f"yT{p}", name=f"yT{p}") for p in range(4)]
            vsb = [None] * NB

            def kq_proj_n(dst, wt, p, n, part=None):
                if part is None or part == 0:
                    ps = psp.tile([128, 512], f32, tag="proj", name="pproj")
                    kq_proj_n.ps = ps
                else:
                    ps = kq_proj_n.ps
                cis = range(4) if part is None else (range(2) if part == 0 else range(2, 4))
                for ci in cis:
                    nc.tensor.matmul(
                        ps[:, :],
                        wt[ci][:, 128 * p:128 * (p + 1)],
                        xt[ci][:, 512 * n:512 * (n + 1)],
                        start=(ci == 0), stop=(ci == 3))
                if part is None or part == 1:
                    nc.vector.tensor_copy(dst[:, 512 * n:512 * (n + 1)], ps[:, :])

            def v_proj_tile(tt, part=None):
                if part is None or part == 0:
                    ps = psp.tile([128, 512], f32, tag="proj", name="pproj")
                    v_proj_tile.ps = ps
                else:
                    ps = v_proj_tile.ps
                cis = range(4) if part is None else (range(2) if part == 0 else range(2, 4))
                for ci in cis:
                    nc.tensor.matmul(
                        ps[:, :],
                        xt[ci][:, 128 * tt:128 * (tt + 1)],
                        wvt[ci][:, :],
                        start=(ci == 0), stop=(ci == 3))
                if part is None or part == 1:
                    # ones-augmented V: [128, 8 heads, 65], col 64 stays 1.0
                    vt = vpool.tile([128, 8, 65], bf16, tag=f"v{tt}", name=f"v{tt}")
                    nc.gpsimd.memset(vt[:, :, 64:65], 1.0)
                    nc.vector.tensor_copy(
                        vt[:, :, 0:64],
                        ps[:, :].rearrange("x (h d) -> x h d", h=8))
                    vsb[tt] = vt

            def out_proj_tile(tt, part=None):
                if part is None or part == 0:
                    ps = psp.tile([128, 512], f32, tag="proj", name="pproj")
                    out_proj_tile.ps = ps
                else:
                    ps = out_proj_tile.ps
                cis = range(4) if part is None else (range(2) if part == 0 else range(2, 4))
                for ci in cis:
                    nc.tensor.matmul(
                        ps[:, :],
                        ytT[ci][:, 128 * tt:128 * (tt + 1)],
                        wpt[ci][:, :],
                        start=(ci == 0), stop=(ci == 3))
                if part is None or part == 1:
                    ot = otp.tile([128, 512], f32, tag="ot", name="ot")
                    nc.vector.tensor_copy(ot[:, :], ps[:, :])
                    nc.sync.dma_start(out=out[128 * tt:128 * (tt + 1), :], in_=ot[:, :])

            def emit_qk(p, qn, j):
                ms_ = max(0, j - 4 * qn)
                trim = 128 * ms_
                qk = qkp.tile([128, 2, 512], f32, tag="qk", name="qk")
                for u in (0, 1):
                    nc.tensor.matmul(
                        qk[:, u, trim:512],
                        kt[p][64 * u:64 * u + 64, 128 * j:128 * (j + 1)],
                        qt[p][64 * u:64 * u + 64, 512 * qn + trim:512 * qn + 512],
                        start=True, stop=True)
                return qk

            def transpose_y(p, qn, ysb):
                # transpose y -> yT (2 heads x 64 = 128 channels per pair)
                # on the (otherwise idle) DMA XBAR; bf16 supports it. One
                # instruction for all 4 q-blocks (HWDGE issue is serialized
                # at ~625ns each): out dim 1 folds into the logical
                # transposed partition dim.
                nc.sync.dma_start_transpose(
                    out=ytT[p][:, 512 * qn:512 * (qn + 1)].rearrange(
                        "x (q c) -> x q c", q=4),
                    in_=ysb[:, :, :, :])

            def drain(pending, budget, now):
                """Emit queued proj work: spend the iteration's spare PE
                budget (ns), then keep going while any queued item's
                deadline is due (FIFO order preserves emission deps)."""
                while pending and (budget > -250
                                   or min(dl for _c, dl, _f in pending) <= now):
                    cost, _dl, fn = pending.popleft()
                    fn()
                    budget -= cost
                return budget

            def attention(p, qn, pending, first_qk, next_pq, bi):
                """Head pair p, q-tile qn. pending: deque of
                (cost, deadline, fn) thunks drained against per-iteration PE
                slack. QK runs one iteration ahead of the PV bursts; the
                NEXT block's first QK is emitted in this block's last
                iteration; the final two PV bursts plus the scale/transpose
                epilogue are deferred into the next block's queue (whose
                early iterations are exp-heavy), so neither engine waits at
                block boundaries."""
                nb = 4 * qn + 4
                # acc[:, u, 65*qb:65*qb+65]: PV accumulator for head 2p+u,
                # q-block qb (col 64 = denominator). One bank per u; at most
                # one open accumulation group per bank at any time.
                acc = acp.tile([128, 2, 512], f32, tag="acc", name="acc")
                pts = [None] * nb
                qk = first_qk
                next_qk = None
                carry = 0.0

                def burst(qb):
                    for u in (0, 1):
                        h = 2 * p + u
                        for j2 in range(0, 4 * qn + qb + 1):
                            nc.tensor.matmul(
                                acc[:, u, 65 * qb:65 * qb + 65],
                                pts[j2][:, u, 128 * qb:128 * (qb + 1)],
                                vsb[j2][:, h, :],
                                start=(j2 == 0), stop=(j2 == 4 * qn + qb))

                def epilogue():
                    # denominators -> reciprocal -> scale+evac -> transpose
                    rec = rcp.tile([128, 2, 4], f32, tag="rec", name="rec")
                    den = bass.AP(tensor=acc.tensor, offset=acc.offset + 64,
                                  ap=[acc.ap[0], [512, 2], [65, 4], [1, 1]])
                    nc.vector.reciprocal(out=rec[:, :, :], in_=den)
                    ysb = ypool.tile([128, 4, 2, 64], bf16, tag="ysb", name="ysb")
                    acc_r = bass.AP(tensor=acc.tensor, offset=acc.offset,
                                    ap=[acc.ap[0], [65, 4], [512, 2], [1, 64]])
                    rec_b = bass.AP(tensor=rec.tensor, offset=rec.offset,
                                    ap=[rec.ap[0], [1, 4], [4, 2], [0, 64]])
                    nc.vector.tensor_mul(ysb[:, :, :, :], acc_r, rec_b)
                    transpose_y(p, qn, ysb)

                for j in range(nb):
                    ms_ = max(0, j - 4 * qn)
                    trim = 128 * ms_
                    pt = ptp.tile([128, 2, 512], bf16, tag="pt", name="pt")
                    nc.scalar.activation(out=pt[:, :, trim:512], in_=qk[:, :, trim:512],
                                         func=EXP, scale=0.125)
                    if j >= 4 * qn:
                        sl = pt[:, :, trim:trim + 128]
                        nc.vector.tensor_mul(sl, sl, tri2[:, :, :])
                    pts[j] = pt
                    exp_ns = 2 * (512 - trim) * 0.833 + 400
                    pe_ns = 0.417 * (1024 - trim)          # next QK
                    if j + 1 < nb:
                        qk = emit_qk(p, qn, j + 1)
                    elif next_pq is not None:
                        next_qk = emit_qk(next_pq[0], next_pq[1], 0)
                    qb_d = j - 4 * qn
                    if 0 <= qb_d <= 0:
                        pe_ns += 0.417 * 65 * 2 * (j + 1)  # inline burst
                    carry = min(drain(pending, exp_ns - pe_ns + min(carry, 0.0),
                                      bi + (j + 1) / 100.0), 1000.0)
                    if 0 <= qb_d <= 0:
                        burst(qb_d)
                    elif qb_d >= 1:
                        cost = int(0.417 * 65 * 2 * (j + 1))
                        pending.append((cost, bi + 2, lambda qb=qb_d: burst(qb)))
                while pending and min(dl for _c, dl, _f in pending) <= bi + 1:
                    pending.popleft()[2]()
                pending.append((300, bi + 2.0, epilogue))
                return next_qk

            # ---- main pipeline
            from collections import deque
            pend = deque()
            # upfront: pair-0 projections for qn 0, first QK, v tiles 0..3
            kq_proj_n(kt[0], wkt, 0, 0)
            kq_proj_n(qt[0], wqt, 0, 0)
            cur_qk = emit_qk(0, 0, 0)
            for tt in range(4):
                v_proj_tile(tt)
            for part in (0, 1):
                pend.append((430, 1.0, lambda part=part: kq_proj_n(kt[1], wkt, 1, 0, part)))
            for part in (0, 1):
                pend.append((430, 1.0, lambda part=part: kq_proj_n(qt[1], wqt, 1, 0, part)))
            blocks = [(qn, p) for qn in range(NT) for p in range(4)]
            for bi, (qn, p) in enumerate(blocks):
                # queue work consumed by LATER attention blocks; drained
                # against per-iteration PE slack, flushed at its deadline
                if bi + 2 < len(blocks):
                    nq2, np2 = blocks[bi + 2]
                    for part in (0, 1):
                        pend.append((430, bi + 2.0, lambda p2=np2, n2=nq2, part=part: kq_proj_n(kt[p2], wkt, p2, n2, part)))
                    for part in (0, 1):
                        pend.append((430, bi + 2.0, lambda p2=np2, n2=nq2, part=part: kq_proj_n(qt[p2], wqt, p2, n2, part)))
                if p == 0 and qn > 0:
                    for tt in range(4 * qn, 4 * qn + 4):
                        # vsb[4qn] is read by this block's inline qb0 burst at
                        # iteration 4qn; the rest only by deferred bursts that
                        # sit behind these items in FIFO order.
                        dl = bi + (4 * qn + 1) / 100.0 if tt == 4 * qn else bi + 0.9
                        for part in (0, 1):
                            pend.append((430, dl,
                                         lambda tt=tt, part=part: v_proj_tile(tt, part)))
                if qn > 0:
                    for part in (0, 1):
                        pend.append((430, bi + 6.0, lambda tt=4 * (qn - 1) + p, part=part: out_proj_tile(tt, part)))
                nxt = None
                if bi + 1 < len(blocks):
                    nq, np_ = blocks[bi + 1]
                    nxt = (np_, nq)
                cur_qk = attention(p, qn, pend, cur_qk, nxt, bi)
            while pend:
                pend.popleft()[2]()
            for tt in range(12, 16):
                out_proj_tile(tt)

    nc.compile()
    return nc


_NC = None


def _get_nc():
    global _NC
    if _NC is None:
        _NC = build_nc()
    return _NC


def kernel(**inputs: np.ndarray) -> np.ndarray:
    x = np.asarray(inputs["x"], dtype=np.float32)
    wqT = np.ascontiguousarray(np.asarray(inputs["Wq"], np.float32).T).astype(BF)
    wkT = np.ascontiguousarray(np.asarray(inputs["Wk"], np.float32).T).astype(BF)
    wvT = np.ascontiguousarray(np.asarray(inputs["Wv"], np.float32).T).astype(BF)
    wpT = np.ascontiguousarray(np.asarray(inputs["Wp"], np.float32).T).astype(BF)
    nc = _get_nc()
    in_maps = []
    for b in range(N_CORES):
        in_maps.append({
            "xT": np.ascontiguousarray(x[b].T).astype(BF),
            "wqT": wqT, "wkT": wkT, "wvT": wvT, "wpT": wpT,
        })
    res = run_bass_kernel_spmd(nc, in_maps, core_ids=list(range(N_CORES)))
    return np.stack([res.results[b]["out"] for b in range(N_CORES)], axis=0)


if __name__ == "__main__":
    nc = _get_nc()
    from concourse.timeline_sim import TimelineSim
    print("TimelineSim predicted ns:", TimelineSim(nc).simulate())
